# revision 4
# baseline (speedup 1.0000x reference)
"""DiffJPEG Trainium2 Bass kernel, v11.

Changes vs v2:
  - input layout (s c w): flat 2-dim pool APs on gpsimd
  - stage1-Y even/odd accumulation split removed: SBUF->SBUF DMA
    shuffles yt rowpair layout into one-row-per-partition tiles, so
    stage1 Y is 4 single matmuls instead of 8 accumulating ones
  - chroma k-pairing: all chroma tiles are [P,512] (k-major), halving
    chroma instruction counts on DVE/ACT/PE/GPSIMD
  - T2 transposes in f32r (1.5 cyc/row vs 2.0)
  - 3-stage skew: front / mid / back interleaved across images
"""

import math
import os
import re

import numpy as np

import concourse.bacc as bacc
import concourse.bass as bass
import concourse.mybir as mybir
from concourse.mybir import ActivationFunctionType as Act, AluOpType as Op
from concourse.tile import TileContext

import bass_rust as _br

# --------------------------------------------------------------------------
# custom DVE ops
# --------------------------------------------------------------------------
import concourse.dve_ops as dve_ops
from concourse.dve_spec import C0, One, Spec, Src0, Src1, Zero, maxx, minn

MAGIC = float(np.float32(1.5 * 2**23))


def _diffround_ref(in0, in1, s0, s1, imm2):
    m = (in0.astype(np.float32) * in1.astype(np.float32)).astype(np.float32)
    r = ((m + np.float32(s0)) - np.float32(s0)).astype(np.float32)
    e = (m - r).astype(np.float32)
    return (r + e * e * e).astype(np.float32)


_m = Src0 * Src1
_r = (_m + C0) - C0
_e = _m - _r
_DR_SPEC = Spec(body=_r + _e * _e * _e, reference=_diffround_ref)


def _clipstt_ref(in0, in1, s0, s1, imm2):
    m = (in0.astype(np.float32) * np.float32(s0) + in1.astype(np.float32)).astype(
        np.float32
    )
    return np.clip(m, 0.0, 1.0).astype(np.float32)


_CS_SPEC = Spec(
    body=maxx(Zero, minn(One, Src0 * C0 + Src1)), reference=_clipstt_ref
)


def _register(name, spec):
    for op in dve_ops.OPS:
        if op.name == name:
            return op
    op = dve_ops.DveOp(name, spec, subdim=False, uops_sha={})
    dve_ops.OPS.append(op)
    dve_ops._SUB_OPCODE_FOR_NAME[name] = (
        dve_ops._CUSTOM_DVE_ROW_BASE + len(dve_ops.OPS) - 1
    )
    dve_ops.CUSTOM_DVE_SPECS[name] = spec
    for ver in ("v3", "v4"):
        try:
            op.compile(ver)
        except ValueError as e:
            m = re.search(r'="([0-9a-f]+)"', str(e))
            if m is None:
                raise
            op.uops_sha[ver] = m.group(1)
            op.compile(ver)
    return op


DIFFROUND = _register("DIFF_ROUND_QANT", _DR_SPEC)
CLIPSTT = _register("CLIP_STT_01", _CS_SPEC)

# --------------------------------------------------------------------------
# constants
# --------------------------------------------------------------------------
P = 128
DT = mybir.dt.float32
BF = mybir.dt.bfloat16
NIMG = 4
FACTOR = 0.4
F32R_INV = os.environ.get("KERNEL_F32R_INV", "1") == "1"
T2R = os.environ.get("KERNEL_T2R", "1") == "1"


def _mk_layout(items):
    off_map, off = {}, 0
    for n, w in items:
        off_map[n] = (off, w)
        off += w
    return off_map, off


_CONST_OFF, _CTOT = _mk_layout(
    [
        ("ident", 128),
        ("q1y", 512),
        ("p2y", 512),
        ("q1c", 512),
        ("p2c", 512),
        ("bias_c1y", 1),
        ("bias_c4y", 1),
    ]
)
_CONSTF_OFF, _CFTOT = _mk_layout(
    [("w_s1r", 128), ("w_s1c", 128), ("w_s2", 128)]
)
_CONSTI_OFF, _CITOT = _mk_layout(
    [("w_idct", 128), ("w_ibc0", 128), ("w_ibc1", 128), ("identr", 128)]
)

_AY = 0.587 / 0.299
_BY = 0.114 / 0.587
_ACB = -0.331264 / 0.5
_BCB = -0.168736 / 0.5
_RCB = _BCB / _ACB
_ACR = -0.418688 / 0.5
_BCR = -0.081312 / 0.5
_RCR = _BCR / _ACR


def build_const_arrays(y_table, c_table):
    A = np.zeros((8, 8), np.float64)
    for u in range(8):
        for x in range(8):
            A[u, x] = math.cos((2 * x + 1) * u * math.pi / 16)
    alpha = np.array([1.0 / math.sqrt(2)] + [1.0] * 7)
    Ah = (0.5 * alpha)[:, None] * A
    cY = 255.0 * 0.299
    cC = 0.5 * 255.0 / 4.0

    C = {}
    # stage1-Y row layout: partitions = 16 block-rows x 8 rows
    W = np.zeros((128, 128))
    for pr in range(128):
        Ib, x = pr // 8, pr % 8
        for u in range(8):
            W[pr, 8 * Ib + u] = Ah[u, x] * cY
    C["w_s1r"] = W
    W = np.zeros((128, 128))
    for pr in range(128):
        Ib, x = pr // 8, pr % 8
        for u in range(8):
            W[pr, 8 * Ib + u] = Ah[u, x] * cC
    C["w_s1c"] = W
    W = np.zeros((128, 128))
    for wl in range(128):
        J, y = wl // 8, wl % 8
        for v in range(8):
            W[wl, 8 * J + v] = Ah[v, y]
    C["w_s2"] = W
    W = np.zeros((128, 128))
    for j in range(16):
        for v in range(8):
            for y in range(8):
                W[8 * j + v, 8 * j + y] = Ah[v, y]
    C["w_idct"] = W
    for par in (0, 1):
        W = np.zeros((128, 128))
        for p in range(128):
            xloc = 64 * par + p // 2
            Ib, x = xloc // 8, xloc % 8
            for u in range(8):
                W[8 * Ib + u, p] = Ah[u, x]
        C[f"w_ibc{par}"] = W
    C["ident"] = np.eye(128)
    C["identr"] = np.eye(128)

    def pats(T, ncols):
        T = np.asarray(T, np.float64)
        q1 = np.zeros((128, ncols))
        p2 = np.zeros((128, ncols))
        for p in range(128):
            v = p % 8
            for c in range(ncols):
                u = c % 8
                q1[p, c] = 1.0 / (T[u, v] * FACTOR)
                p2[p, c] = T[u, v] * FACTOR / 255.0
        return q1, p2

    C["q1y"], C["p2y"] = pats(y_table, 512)
    C["q1c"], C["p2c"] = pats(c_table, 512)

    b = np.zeros((128, 1))
    b[0::8, 0] = -1024.0 * 0.5 * alpha[0]
    C["bias_c1y"] = b
    b = np.zeros((128, 1))
    b[0::8, 0] = (128.0 / 255.0) / (0.5 * alpha[0])
    C["bias_c4y"] = b

    def pack(off_map, tot):
        p = np.zeros((128, tot), np.float32)
        for n, (off, w) in off_map.items():
            p[:, off : off + w] = np.asarray(C[n], np.float32)
        return p

    return pack(_CONST_OFF, _CTOT), pack(_CONSTF_OFF, _CFTOT), pack(_CONSTI_OFF, _CITOT)


# --------------------------------------------------------------------------
# program
# --------------------------------------------------------------------------
def build_program():
    FDT = DT
    IDT = mybir.dt.float32r if F32R_INV else DT
    TDT = IDT if T2R else DT  # dtype for c3 tiles / T2 transposes
    nc = bacc.Bacc("TRN2", target_bir_lowering=False)
    img = nc.dram_tensor("img", [NIMG, 3, 512, 512], DT, kind="ExternalInput")
    out = nc.dram_tensor("out", [NIMG, 3, 512, 512], BF, kind="ExternalOutput")
    cdram = nc.dram_tensor("consts", [128, _CTOT], DT, kind="ExternalInput")
    cfdram = nc.dram_tensor("constsf", [128, _CFTOT], FDT, kind="ExternalInput")
    cidram = nc.dram_tensor("constsi", [128, _CITOT], IDT, kind="ExternalInput")

    def dup2(ap):
        return _br.AP(
            tensor=ap.tensor,
            offset=ap.offset,
            ap=[list(ap.ap[0]), list(ap.ap[1]), [0, 2]],
        )

    _GR = 0.344136 / 0.714136

    with TileContext(nc) as tc:
        with (
            tc.tile_pool(name="pc", bufs=1) as pc,
            tc.tile_pool(name="ps", bufs=6, space="PSUM") as ps,
            tc.tile_pool(name="psc", bufs=2, space="PSUM") as psc,
            tc.tile_pool(name="pin", bufs=2) as pin,
            tc.tile_pool(name="py", bufs=2) as py,
            tc.tile_pool(name="pyr", bufs=6) as pyr,
            tc.tile_pool(name="php", bufs=1) as php,
            tc.tile_pool(name="pwp", bufs=2) as pwp,
            tc.tile_pool(name="pcc", bufs=2) as pcc,
            tc.tile_pool(name="pst1", bufs=4) as pst1,
            tc.tile_pool(name="pt2s", bufs=4) as pt2s,
            tc.tile_pool(name="pmid", bufs=4) as pmid,
            tc.tile_pool(name="pdeq", bufs=10) as pdeq,
            tc.tile_pool(name="pc3", bufs=6) as pc3,
            tc.tile_pool(name="pc4", bufs=4) as pc4,
            tc.tile_pool(name="pcup", bufs=4) as pcup,
            tc.tile_pool(name="prgb", bufs=6) as prgb,
        ):
            cwt = pc.tile([128, _CTOT], DT, tag="consts", name="t_consts")
            nc.sync.dma_start(out=cwt[:], in_=cdram[:])
            cwtf = pc.tile([128, _CFTOT], FDT, tag="constsf", name="t_constsf")
            nc.sync.dma_start(out=cwtf[:], in_=cfdram[:])
            cwti = pc.tile([128, _CITOT], IDT, tag="constsi", name="t_constsi")
            nc.sync.dma_start(out=cwti[:], in_=cidram[:])
            cw = {n: cwt[:, off : off + w] for n, (off, w) in _CONST_OFF.items()}
            cw.update(
                {n: cwtf[:, off : off + w] for n, (off, w) in _CONSTF_OFF.items()}
            )
            cw.update(
                {n: cwti[:, off : off + w] for n, (off, w) in _CONSTI_OFF.items()}
            )
            scr = pc.tile([1, 8], DT, tag="scr", name="scr0")
            nc.vector.tensor_copy(scr[0:1, 0:1], cwt[0:1, 0:1])
            nc.scalar.activation(scr[0:1, 1:2], cwt[0:1, 0:1], Act.Copy)

            S = {}

            def front(b):
                # ---- load (s c w) rowpair tiles ----
                planes = img[b].rearrange("c (rp s) w -> rp s c w", s=2)
                tiles = {}
                for k in range(2):
                    t = pin.tile([P, 3072], DT, tag="in", name=f"in{b}_{k}")
                    nc.sync.dma_start(
                        out=t[:].rearrange("p (s c f) -> p s c f", s=2, c=3),
                        in_=planes[128 * k : 128 * k + 128],
                    )
                    tiles[k] = t

                # ---- Y color + row-shuffle ----
                ytr = {}
                for k in range(2):
                    t = tiles[k]

                    def chan(c):  # [p, s(2), w(512)] view of channel c
                        return t[:].rearrange("p (s f) -> p s f", s=2)[
                            :, :, 512 * c : 512 * c + 512
                        ]

                    t1 = py.tile([P, 1024], DT, tag="yt1", bufs=1, name=f"yt1_{b}{k}")
                    t1v = t1[:].rearrange("p (s w) -> p s w", s=2)
                    nc.vector.scalar_tensor_tensor(
                        t1v, chan(2), _BY, chan(1), Op.mult, Op.add
                    )
                    ytk = py.tile([P, 1024], FDT, tag="yt", bufs=4, name=f"yt{b}_{k}")
                    nc.vector.scalar_tensor_tensor(
                        ytk[:].rearrange("p (s w) -> p s w", s=2),
                        t1v,
                        _AY,
                        chan(0),
                        Op.mult,
                        Op.add,
                    )
                    for half in range(2):
                        m = 2 * k + half
                        r = pyr.tile([P, 512], FDT, tag="ytr", name=f"ytr{b}_{m}")
                        nc.sync.dma_start(
                            out=r[:].rearrange("p (s w) -> p s w", s=64),
                            in_=ytk[64 * half : 64 * half + 64].rearrange(
                                "p (s w) -> p s w", s=2
                            ),
                        )
                        ytr[m] = r

                # ---- 2x2 pooling (flat APs) ----
                wpimg = pwp.tile([P, 1536], DT, tag="wp", name=f"wp{b}")
                for k in range(2):
                    t = tiles[k]
                    hp = php.tile([P, 1536], DT, tag="hp", name=f"hp{b}_{k}")
                    nc.gpsimd.tensor_tensor(
                        hp[:], t[:, 0:1536], t[:, 1536:3072], Op.add
                    )
                    nc.gpsimd.tensor_tensor(
                        wpimg[:, 768 * k : 768 * k + 768],
                        hp[:, 0:1536:2],
                        hp[:, 1:1536:2],
                        Op.add,
                    )

                # ---- chroma color (k-paired [512]) ----
                def pview(c):  # [p, k(2), w(256)] view of pooled channel c
                    return wpimg[:].rearrange("p (k f) -> p k f", k=2)[
                        :, :, 256 * c : 256 * c + 256
                    ]

                cbt = pcc.tile([P, 512], DT, tag="cct", name=f"cbt{b}")
                cbtv = cbt[:].rearrange("p (k w) -> p k w", k=2)
                nc.vector.scalar_tensor_tensor(
                    cbtv, pview(0), _RCB, pview(1), Op.mult, Op.add
                )
                cb = pcc.tile([P, 512], FDT, tag="cb", name=f"cb{b}")
                nc.vector.scalar_tensor_tensor(
                    cb[:].rearrange("p (k w) -> p k w", k=2),
                    cbtv,
                    _ACB,
                    pview(2),
                    Op.mult,
                    Op.add,
                )
                crt = pcc.tile([P, 512], DT, tag="cct", name=f"crt{b}")
                crtv = crt[:].rearrange("p (k w) -> p k w", k=2)
                nc.vector.scalar_tensor_tensor(
                    crtv, pview(2), _RCR, pview(1), Op.mult, Op.add
                )
                cr = pcc.tile([P, 512], FDT, tag="cr", name=f"cr{b}")
                nc.vector.scalar_tensor_tensor(
                    cr[:].rearrange("p (k w) -> p k w", k=2),
                    crtv,
                    _ACR,
                    pview(0),
                    Op.mult,
                    Op.add,
                )
                S[b] = (ytr, {"cb": cb, "cr": cr})

            def mid(b):
                ytr, cbcr = S[b]
                st1 = {}
                # ---- stage 1 Y: 4 single matmuls on row-shuffled tiles ----
                for m in range(4):
                    pt = ps.tile([P, 512], DT, tag="ps", name=f"p_s1y{b}{m}")
                    nc.tensor.matmul(
                        pt[:], cw["w_s1r"][:], ytr[m][:], start=True, stop=True
                    )
                    s = pst1.tile([P, 512], DT, tag="st1", name=f"st1y{b}{m}")
                    nc.scalar.activation(
                        s[:], pt[:], Act.Identity, bias=cw["bias_c1y"][:, 0:1]
                    )
                    st1["y", m] = s
                # ---- stage 1 chroma: one matmul per channel ----
                for ch in ("cb", "cr"):
                    pt = ps.tile([P, 512], DT, tag="ps", name=f"p_s1{ch}{b}")
                    nc.tensor.matmul(
                        pt[:], cw["w_s1c"][:], cbcr[ch][:], start=True, stop=True
                    )
                    s = pst1.tile([P, 512], DT, tag="st1c", bufs=2, name=f"st1{ch}{b}")
                    nc.scalar.activation(s[:], pt[:], Act.Copy)
                    st1[ch] = s

                # ---- T1 transpose + c2 ----
                t2s = {}
                for j in range(4):
                    pt = ps.tile([P, 512], DT, tag="ps", name=f"p_t1y{b}{j}")
                    for m in range(4):
                        nc.tensor.transpose(
                            pt[:, 128 * m : 128 * m + 128],
                            st1["y", m][:, 128 * j : 128 * j + 128],
                            cw["ident"][:],
                        )
                    s = pt2s.tile([P, 512], FDT, tag="t2s", name=f"t2sy{b}{j}")
                    nc.scalar.activation(s[:], pt[:], Act.Copy)
                    t2s["y", j] = s
                for ch in ("cb", "cr"):
                    pt = ps.tile([P, 512], DT, tag="ps", name=f"p_t1{ch}{b}")
                    for jc in range(2):
                        for k in range(2):
                            nc.tensor.transpose(
                                pt[:, 256 * jc + 128 * k : 256 * jc + 128 * k + 128],
                                st1[ch][:, 256 * k + 128 * jc : 256 * k + 128 * jc + 128],
                                cw["ident"][:],
                            )
                    s = pt2s.tile([P, 512], FDT, tag="t2sc", bufs=2, name=f"t2s{ch}{b}")
                    nc.scalar.activation(s[:], pt[:], Act.Copy)
                    t2s[ch] = s

                # ---- stage 2 + quant/diff_round/dequant ----
                deq = {}
                for key, q1, p2, nj in (
                    ("y", "q1y", "p2y", 4),
                    ("cb", "q1c", "p2c", 1),
                    ("cr", "q1c", "p2c", 1),
                ):
                    for j in range(nj):
                        mv = t2s[key, j] if key == "y" else t2s[key]
                        pt = ps.tile([P, 512], DT, tag="ps", name=f"p_s2{key}{b}{j}")
                        nc.tensor.matmul(
                            pt[:], cw["w_s2"][:], mv[:], start=True, stop=True
                        )
                        ymid = pmid.tile([P, 512], DT, tag="ymid", name=f"md{key}{b}{j}")
                        nc.vector._custom_dve(
                            DIFFROUND,
                            out=ymid[:],
                            in0=pt[:],
                            in1=cw[q1][:, 0:512],
                            s0=MAGIC,
                        )
                        d = pdeq.tile([P, 512], IDT, tag="deq", name=f"dq{key}{b}{j}")
                        nc.gpsimd.tensor_tensor(
                            d[:], ymid[:], cw[p2][:, 0:512], Op.mult
                        )
                        deq[key, j] = d
                S[b] = (deq,)

            def back(b):
                (deq,) = S.pop(b)
                # ---- iA (inverse W) + c3 ----
                c3 = {}
                for key, nj in (("y", 4), ("cb", 1), ("cr", 1)):
                    for j in range(nj):
                        pt = ps.tile([P, 512], DT, tag="ps", name=f"p_ia{key}{b}{j}")
                        nc.tensor.matmul(
                            pt[:], cw["w_idct"][:], deq[key, j][:], start=True, stop=True
                        )
                        s = pc3.tile([P, 512], TDT, tag="c3", name=f"c3{key}{b}{j}")
                        nc.scalar.activation(s[:], pt[:], Act.Copy)
                        c3[key, j] = s

                tident = cw["identr"] if T2R else cw["ident"]
                # ---- T2 transpose + c4 ----
                c4 = {}
                for m in range(4):
                    pt = ps.tile([P, 512], TDT, tag="ps", name=f"p_t2y{b}{m}")
                    for j in range(4):
                        nc.tensor.transpose(
                            pt[:, 128 * j : 128 * j + 128],
                            c3["y", j][:, 128 * m : 128 * m + 128],
                            tident[:],
                        )
                    s = pc4.tile([P, 512], IDT, tag="c4", name=f"c4y{b}{m}")
                    nc.scalar.activation(
                        s[:], pt[:], Act.Identity, bias=cw["bias_c4y"][:, 0:1]
                    )
                    c4["y", m] = s
                for ch in ("cb", "cr"):
                    pt = ps.tile([P, 512], TDT, tag="ps", name=f"p_t2{ch}{b}")
                    for k in range(2):
                        for jc in range(2):
                            nc.tensor.transpose(
                                pt[:, 256 * k + 128 * jc : 256 * k + 128 * jc + 128],
                                c3[ch, 0][:, 256 * jc + 128 * k : 256 * jc + 128 * k + 128],
                                tident[:],
                            )
                    s = pc4.tile([P, 512], IDT, tag="c4c", bufs=2, name=f"c4{ch}{b}")
                    nc.scalar.activation(s[:], pt[:], Act.Copy)
                    c4[ch] = s

                # ---- iB + upsample + recombine (fused clip, bf16) + store ----
                for mo in range(4):
                    ypt = ps.tile([P, 512], DT, tag="ps", name=f"p_iby{b}{mo}")
                    nc.tensor.matmul(
                        ypt[:], cw["w_idct"][:], c4["y", mo][:], start=True, stop=True
                    )
                    cq = {}
                    for ch in ("cb", "cr"):
                        cpt = psc.tile([P, 256], DT, tag="psc", name=f"p_ib{ch}{b}{mo}")
                        nc.tensor.matmul(
                            cpt[:],
                            cw[f"w_ibc{mo % 2}"][:],
                            c4[ch][:, 256 * (mo // 2) : 256 * (mo // 2) + 256],
                            start=True,
                            stop=True,
                        )
                        q = pcup.tile([P, 256], DT, tag="cup", name=f"cu{ch}{b}{mo}")
                        nc.scalar.activation(q[:], cpt[:], Act.Copy)
                        cq[ch] = q

                    rows = slice(128 * mo, 128 * mo + 128)
                    ab = prgb.tile([1, 1], DT, tag="ab", name=f"ab{b}{mo}")
                    nc.vector.tensor_copy(ab[0:1, 0:1], ypt[0:1, 0:1])
                    rgb = prgb.tile([P, 1536], BF, tag="rgb", bufs=3, name=f"rgb{b}{mo}")
                    nc.vector._custom_dve(
                        CLIPSTT,
                        out=rgb[:, 0:512],
                        in0=dup2(cq["cr"][:]),
                        in1=ypt[:],
                        s0=1.402,
                    )
                    gq = pcup.tile([P, 256], DT, tag="gq", bufs=2, name=f"gq{b}{mo}")
                    nc.vector.scalar_tensor_tensor(
                        gq[:], cq["cb"][:], _GR, cq["cr"][:], Op.mult, Op.add
                    )
                    nc.vector._custom_dve(
                        CLIPSTT,
                        out=rgb[:, 512:1024],
                        in0=dup2(gq[:]),
                        in1=ypt[:],
                        s0=-0.714136,
                    )
                    nc.vector._custom_dve(
                        CLIPSTT,
                        out=rgb[:, 1024:1536],
                        in0=dup2(cq["cb"][:]),
                        in1=ypt[:],
                        s0=1.772,
                    )
                    nc.sync.dma_start(
                        out=out[b][:, rows, :].rearrange("c h w -> h c w"),
                        in_=rgb[:].rearrange("p (c f) -> p c f", c=3),
                    )

            # 3-stage skew
            front(0)
            front(1)
            mid(0)
            front(2)
            mid(1)
            back(0)
            front(3)
            mid(2)
            back(1)
            mid(3)
            back(2)
            back(3)

    nc.compile()
    return nc


# --------------------------------------------------------------------------
# entry point
# --------------------------------------------------------------------------
_last_results = None


def kernel(image, y_table, c_table):
    global _last_results
    from concourse import bass_utils

    image = np.ascontiguousarray(np.asarray(image), np.float32)
    packed, packedf, packedi = build_const_arrays(
        np.asarray(y_table), np.asarray(c_table)
    )

    nc = build_program()
    n_cores = 8
    per = image.shape[0] // n_cores
    in_maps = [
        {
            "img": np.ascontiguousarray(image[i * per : (i + 1) * per]),
            "consts": packed,
            "constsf": packedf,
            "constsi": packedi,
        }
        for i in range(n_cores)
    ]

    res = None
    last_exc = None
    for attempt in range(3):
        try:
            res = bass_utils.run_bass_kernel_spmd(
                nc,
                in_maps,
                core_ids=list(range(n_cores)),
                trace=os.environ.get("KERNEL_TRACE", "0") == "1",
            )
            break
        except Exception as e:
            last_exc = e
    if res is None:
        raise last_exc
    _last_results = res
    outs = [np.asarray(r["out"]).astype(np.float32) for r in res.results]
    return np.concatenate(outs, axis=0)


if __name__ == "__main__":
    rng = np.random.default_rng(0)
    img = rng.random((32, 3, 512, 512), np.float32)
    yt = np.ones((8, 8), np.float32)
    ct = np.ones((8, 8), np.float32)
    out = kernel(img, yt, ct)
    print("out", out.shape, out.dtype, float(out.min()), float(out.max()))
